# revision 8
# baseline (speedup 1.0000x reference)
"""GCN layer kernel for nn_GcnNet_17695265259748 — full on-device Bass SPMD.

Pipeline per NeuronCore (8 cores, nodes sharded 6250/core):
  1. stream x shard [6250,20,128] f32, reduce over L on DVE, scale by
     dis/L -> y_local bf16 [6272,128] (22 zero pad rows) in DRAM
  2. AllGather -> y_full [50176,128] bf16 (all cores' scaled features)
  3. per dst tile (49 x 128 dsts): dma_gather edge source rows across the
     4 SWDGE queues (signed idx16 rebased at mid-table so one table covers
     all 50176 rows), one-hot M built on DVE in batches (broadcast
     tensor_tensor, bf16), PSUM aggT += G.T @ M per chunk
  4. proj: psum_out = aggT.T @ W + outer(s', b);  out = dis * psum_out
Host does edge preprocessing (sort by dst, chunking, index tables).
"""

import sys
import numpy as np

for p in ("/opt/trn_rl_repo",):
    if p not in sys.path:
        sys.path.insert(0, p)

N, L, C, F = 50000, 20, 128, 300
NCORES = 8
NPC = N // NCORES            # 6250 nodes per core
NTILES = (NPC + 127) // 128  # 49 dst tiles (last partial: 106)
NPADC = NTILES * 128         # 6272 rows per core chunk in the table
NROWS = NCORES * NPADC       # 50176 table rows
MID = NROWS // 2             # 25088: signed-idx16 rebase point
MAXCALL = 1024               # dma_gather ring limit (rows per call)
NQ = 4                       # SWDGE queues (gpsimd cpu pairs)

_BUILD_CACHE = {}


def _table_row(src):
    return (src // NPC) * NPADC + (src % NPC)


def _preprocess(edge_index):
    """Per-core idx/dstloc tables + uniform per-tile chunk counts."""
    row = np.ascontiguousarray(edge_index[0]).astype(np.int64)
    col = np.ascontiguousarray(edge_index[1]).astype(np.int64)
    keep = row != col
    loops = np.arange(N, dtype=np.int64)
    srcs = np.concatenate([row[keep], loops])
    dsts = np.concatenate([col[keep], loops])

    deg = np.bincount(row[keep], minlength=N).astype(np.float64) + 1.0
    dis = (deg ** -0.5).astype(np.float32)
    sprime = np.bincount(dsts, weights=dis[srcs].astype(np.float64), minlength=N)
    sprime = sprime.astype(np.float32)

    order = np.argsort(dsts, kind="stable")
    ds = dsts[order]
    sr = srcs[order]
    trow = _table_row(sr) - MID  # rebased signed index, in [-25088, 25087]

    core_of = ds // NPC
    tloc = (ds - core_of * NPC) // 128
    gt = core_of * NTILES + tloc
    cnt = np.bincount(gt, minlength=NCORES * NTILES)
    CH = ((cnt.reshape(NCORES, NTILES) + 127) // 128).max(axis=0)  # uniform SPMD
    np.maximum(CH, 1, out=CH)
    CHTOT = int(CH.sum())
    tile_base = np.concatenate([[0], np.cumsum(CH)])[:-1]

    per_core = []
    for c in range(NCORES):
        m = core_of == c
        dsc = ds[m]
        src_r = trow[m]
        dloc = dsc - c * NPC
        tl = dloc // 128
        dl = dloc % 128

        idx = np.zeros((128, CHTOT), dtype=np.int32)  # pad -> row MID (safe)
        dstloc = np.full((128, CHTOT), -1.0, dtype=np.float32)

        # rank of edge within its tile group (stable dst-sorted order)
        korder = np.argsort(tl, kind="stable")
        ks = tl[korder]
        grp_start = np.searchsorted(ks, np.arange(NTILES))
        rank = np.arange(len(ks)) - grp_start[ks]
        chunk = tile_base[ks] + rank // 128
        pos = rank % 128
        idx[pos, chunk] = src_r[korder]
        dstloc[pos, chunk] = dl[korder].astype(np.float32)

        # dma_gather ucode drops TRAILING negative idxs per call; rebased
        # idxs are negative for the lower table half. Ensure the last slot
        # of every call batch is >= 0 by swapping within the batch.
        for t in range(NTILES):
            b0 = tile_base[t]
            done = 0
            while done < CH[t]:
                take = min(CH[t] - done, 8)
                c0 = b0 + done
                clast = c0 + take - 1
                if idx[127, clast] < 0:
                    blk = idx[:, c0 : clast + 1]
                    pp, cc = np.nonzero(blk >= 0)
                    assert len(pp), "all-negative gather call"
                    p2, c2 = pp[0], c0 + cc[0]
                    idx[127, clast], idx[p2, c2] = idx[p2, c2], idx[127, clast]
                    dstloc[127, clast], dstloc[p2, c2] = (
                        dstloc[p2, c2],
                        dstloc[127, clast],
                    )
                done += take

        # idx16: flat slot i = chunk*128 + p -> tile (i%16, i//16)
        flat = idx.T.reshape(-1)
        assert flat.min() >= -32768 and flat.max() < 32768
        idx16 = flat.astype(np.int16).reshape(-1, 16).T.copy()

        per_core.append(
            {
                "idx16": idx16,
                "dstloc": dstloc,
                "dis": dis[c * NPC : (c + 1) * NPC],
                "sprime": sprime[c * NPC : (c + 1) * NPC],
            }
        )
    return per_core, CH, CHTOT, dis


def _build(structure):
    """Build the SPMD Bass program. structure = tuple(CH)."""
    key = structure
    if key in _BUILD_CACHE:
        return _BUILD_CACHE[key]

    import concourse.bass as bass
    import concourse.bacc as bacc
    import concourse.mybir as mybir
    import concourse.tile as tile

    CH = list(structure)
    CHTOT = sum(CH)
    maxch = max(CH)

    nc = bacc.Bacc(
        None, target_bir_lowering=False, debug=False, num_swdge_queues=NQ
    )
    x_in = nc.dram_tensor("x", [NPC, C, L], mybir.dt.float32, kind="ExternalInput")
    dsc_in = nc.dram_tensor("dscale", [128, NTILES], mybir.dt.float32, kind="ExternalInput")
    disv_in = nc.dram_tensor("disv", [128, NTILES], mybir.dt.float32, kind="ExternalInput")
    sp_in = nc.dram_tensor("sp", [1, NPADC], mybir.dt.float32, kind="ExternalInput")
    idx_in = nc.dram_tensor("idx", [16, CHTOT * 8], mybir.dt.int16, kind="ExternalInput")
    dl_in = nc.dram_tensor("dstloc", [128, CHTOT, 1], mybir.dt.bfloat16, kind="ExternalInput")
    iota_in = nc.dram_tensor("iota", [128, 8, 128], mybir.dt.bfloat16, kind="ExternalInput")
    w_in = nc.dram_tensor("W", [C, F], mybir.dt.float32, kind="ExternalInput")
    b_in = nc.dram_tensor("b", [1, F], mybir.dt.float32, kind="ExternalInput")
    out = nc.dram_tensor("out", [NPADC, F], mybir.dt.float32, kind="ExternalOutput")

    with tile.TileContext(nc) as tc:
        with (
            tc.tile_pool(name="sb", bufs=2) as sb,
            tc.tile_pool(name="sbg", bufs=6) as sbg,
            tc.tile_pool(name="sbm", bufs=6) as sbm,
            tc.tile_pool(name="sbx", bufs=2) as sbx,
            tc.tile_pool(name="ps", bufs=4, space="PSUM") as ps,
            tc.tile_pool(name="pso", bufs=4, space="PSUM") as pso,
            tc.tile_pool(name="dram", bufs=1, space="DRAM") as dram,
        ):
            y_loc = dram.tile([NPADC, C], mybir.dt.bfloat16)
            y_full = dram.tile([NROWS, C], mybir.dt.bfloat16)

            # ---- constants ----
            dsc = sb.tile([128, NTILES], mybir.dt.float32, tag="dsc")
            nc.sync.dma_start(dsc[:], dsc_in[:])
            disv = sb.tile([128, NTILES], mybir.dt.float32, tag="disv")
            nc.sync.dma_start(disv[:], disv_in[:])
            spv = sb.tile([1, NPADC], mybir.dt.float32, tag="spv")
            nc.sync.dma_start(spv[:], sp_in[:])
            spb = sb.tile([1, NPADC], mybir.dt.bfloat16, tag="spb")
            nc.vector.tensor_copy(spb[:], spv[:])
            iot8 = sb.tile([128, 8, 128], mybir.dt.bfloat16, tag="iot8")
            nc.sync.dma_start(iot8[:], iota_in[:])
            w32 = sb.tile([128, F], mybir.dt.float32, tag="w32")
            nc.sync.dma_start(w32[:], w_in[:])
            wb = sb.tile([128, F], mybir.dt.bfloat16, tag="wb")
            nc.vector.tensor_copy(wb[:], w32[:])
            b32 = sb.tile([1, F], mybir.dt.float32, tag="b32")
            nc.sync.dma_start(b32[:], b_in[:])
            bb = sb.tile([1, F], mybir.dt.bfloat16, tag="bb")
            nc.vector.tensor_copy(bb[:], b32[:])

            # ---- phase 3 index tables (loaded early, independent of AG) ----
            it = sb.tile([128, CHTOT * 8], mybir.dt.int16, tag="it")
            for k in range(8):
                nc.gpsimd.dma_start(it[16 * k : 16 * (k + 1), :], idx_in[:])
            dltb = sb.tile([128, CHTOT, 1], mybir.dt.bfloat16, tag="dltb")
            nc.gpsimd.dma_start(dltb[:], dl_in[:])

            # ---- phase 1: mean over L, scale, write y_local ----
            for t in range(NTILES):
                n0 = t * 128
                n1 = min(NPC, n0 + 128)
                nn = n1 - n0
                xt = sbx.tile([128, C, L], mybir.dt.float32, tag="xt")
                dma_eng = (nc.sync, nc.gpsimd, nc.scalar, nc.gpsimd)[t % 4]
                dma_eng.dma_start(xt[:nn], x_in[n0:n1])
                xs = sb.tile([128, C], mybir.dt.float32, tag="xs")
                nc.vector.reduce_sum(
                    xs[:nn], xt[:nn], axis=mybir.AxisListType.X,
                )
                yb = sb.tile([128, C], mybir.dt.bfloat16, tag="yb")
                if nn < 128:
                    nc.vector.memset(yb[:], 0.0)
                nc.vector.tensor_scalar_mul(yb[:nn], xs[:nn], dsc[:nn, t : t + 1])
                (nc.sync if t % 2 == 0 else nc.scalar).dma_start(
                    y_loc[n0 : n0 + 128], yb[:]
                )

            # ---- phase 2: AllGather ----
            nc.gpsimd.collective_compute(
                "AllGather",
                mybir.AluOpType.bypass,
                replica_groups=[list(range(NCORES))],
                ins=[y_loc.opt()],
                outs=[y_full.opt()],
            )

            # ---- phase 3: gather + one-hot scatter matmuls ----
            qrr = 0
            chbase = 0
            for t in range(NTILES):
                ctot = CH[t]
                pt = ps.tile([128, 128], mybir.dt.float32)
                done = 0
                while done < ctot:
                    take = min(ctot - done, 8)
                    c0 = chbase + done
                    gb = sbg.tile([128, 8, C], mybir.dt.bfloat16, tag="g")
                    nc.gpsimd.dma_gather(
                        gb[:, :take, :],
                        y_full[MID:, :],
                        it[:, c0 * 8 : (c0 + take) * 8],
                        take * 128,
                        take * 128,
                        C,
                        queue_num=qrr % NQ,
                    )
                    qrr += 1
                    mb = sbm.tile([128, 8, 128], mybir.dt.bfloat16, tag="m")
                    nc.vector.tensor_tensor(
                        out=mb[:, :take, :],
                        in0=iot8[:, :take, :],
                        in1=dltb[:, c0 : c0 + take, :].to_broadcast([128, take, 128]),
                        op=mybir.AluOpType.is_equal,
                    )
                    for ch in range(take):
                        nc.tensor.matmul(
                            pt[:],
                            gb[:, ch, :],
                            mb[:, ch, :],
                            start=(done + ch == 0),
                            stop=(done + ch == ctot - 1),
                        )
                    done += take
                aggb = sb.tile([128, 128], mybir.dt.bfloat16, tag="aggb")
                nc.scalar.copy(aggb[:], pt[:])
                po = pso.tile([128, F], mybir.dt.float32)
                nc.tensor.matmul(po[:], aggb[:], wb[:], start=True, stop=False)
                nc.tensor.matmul(
                    po[:],
                    spb[:, t * 128 : (t + 1) * 128],
                    bb[:],
                    start=False,
                    stop=True,
                )
                ot = sb.tile([128, F], mybir.dt.float32, tag="ot")
                nc.scalar.mul(ot[:], po[:], disv[:, t : t + 1])
                nc.sync.dma_start(out[t * 128 : (t + 1) * 128], ot[:])
                chbase += ctot

    nc.finalize()
    _BUILD_CACHE[key] = nc
    return nc


def kernel(x, edge_index, W, b):
    x = np.ascontiguousarray(np.asarray(x, dtype=np.float32))
    edge_index = np.asarray(edge_index)
    W = np.ascontiguousarray(np.asarray(W, dtype=np.float32))
    b = np.ascontiguousarray(np.asarray(b, dtype=np.float32))

    per_core, CH, CHTOT, dis = _preprocess(edge_index)
    nc = _build(tuple(int(v) for v in CH))

    import ml_dtypes

    iota = np.broadcast_to(
        np.arange(128, dtype=np.float32), (128, 8, 128)
    ).astype(ml_dtypes.bfloat16)
    in_maps = []
    for c in range(NCORES):
        pc = per_core[c]
        dpad = np.zeros(NPADC, dtype=np.float32)
        dpad[:NPC] = pc["dis"]
        spad = np.zeros(NPADC, dtype=np.float32)
        spad[:NPC] = pc["sprime"]
        dsc = np.ascontiguousarray((dpad / L).reshape(NTILES, 128).T)
        dsv = np.ascontiguousarray(dpad.reshape(NTILES, 128).T)
        in_maps.append(
            {
                "x": np.ascontiguousarray(
                    x[c * NPC : (c + 1) * NPC].transpose(0, 2, 1)
                ),
                "dscale": dsc,
                "disv": dsv,
                "sp": spad.reshape(1, NPADC),
                "idx": pc["idx16"],
                "dstloc": pc["dstloc"].astype(ml_dtypes.bfloat16)[:, :, None],
                "iota": iota,
                "W": W,
                "b": b.reshape(1, F),
            }
        )

    from concourse.bass_utils import run_bass_kernel_spmd

    res = run_bass_kernel_spmd(nc, in_maps, core_ids=list(range(NCORES)))
    out = np.empty((N, F), dtype=np.float32)
    for c in range(NCORES):
        out[c * NPC : (c + 1) * NPC] = res.results[c]["out"][:NPC]
    return out


# revision 10
# speedup vs baseline: 1.1456x; 1.1456x over previous
"""GCN layer kernel for nn_GcnNet_17695265259748 — full on-device Bass SPMD.

Pipeline per NeuronCore (8 cores, nodes sharded 6250/core):
  1. stream x shard [6250,20,128] f32, reduce over L on DVE, scale by
     dis/L -> y_local bf16 [6272,128] (22 zero pad rows) in DRAM
  2. AllGather -> y_full [50176,128] bf16 (all cores' scaled features)
  3. per dst tile (49 x 128 dsts): dma_gather edge source rows across the
     4 SWDGE queues (signed idx16 rebased at mid-table so one table covers
     all 50176 rows), one-hot M built on DVE in batches (broadcast
     tensor_tensor, bf16), PSUM aggT += G.T @ M per chunk
  4. proj: psum_out = aggT.T @ W + outer(s', b);  out = dis * psum_out
Host does edge preprocessing (sort by dst, chunking, index tables).
"""

import sys
import numpy as np

for p in ("/opt/trn_rl_repo",):
    if p not in sys.path:
        sys.path.insert(0, p)

N, L, C, F = 50000, 20, 128, 300
NCORES = 8
NPC = N // NCORES            # 6250 nodes per core
NTILES = (NPC + 127) // 128  # 49 dst tiles (last partial: 106)
NPADC = NTILES * 128         # 6272 rows per core chunk in the table
NROWS = NCORES * NPADC       # 50176 table rows
MID = NROWS // 2             # 25088: signed-idx16 rebase point
MAXCALL = 1024               # dma_gather ring limit (rows per call)
NQ = 4                       # SWDGE queues (gpsimd cpu pairs)

_BUILD_CACHE = {}


SPLIT = 3200            # phase-1 row boundary for the two AllGather halves
LO_ROWS = NCORES * SPLIT  # 25600


def _table_row(src):
    """Row in y_full given the two-block AllGather layout:
    lo block = concat over cores of y_loc[0:SPLIT], hi block = rest."""
    core = src // NPC
    r = src % NPC
    return np.where(
        r < SPLIT,
        core * SPLIT + r,
        LO_ROWS + core * (NPADC - SPLIT) + (r - SPLIT),
    )


def _preprocess(edge_index):
    """Per-core idx/dstloc tables + uniform per-tile chunk counts."""
    row = np.ascontiguousarray(edge_index[0]).astype(np.int64)
    col = np.ascontiguousarray(edge_index[1]).astype(np.int64)
    keep = row != col
    loops = np.arange(N, dtype=np.int64)
    srcs = np.concatenate([row[keep], loops])
    dsts = np.concatenate([col[keep], loops])

    deg = np.bincount(row[keep], minlength=N).astype(np.float64) + 1.0
    dis = (deg ** -0.5).astype(np.float32)
    sprime = np.bincount(dsts, weights=dis[srcs].astype(np.float64), minlength=N)
    sprime = sprime.astype(np.float32)

    order = np.argsort(dsts, kind="stable")
    ds = dsts[order]
    sr = srcs[order]
    trow = _table_row(sr) - MID  # rebased signed index, in [-25088, 25087]

    core_of = ds // NPC
    tloc = (ds - core_of * NPC) // 128
    gt = core_of * NTILES + tloc
    cnt = np.bincount(gt, minlength=NCORES * NTILES)
    CH = ((cnt.reshape(NCORES, NTILES) + 127) // 128).max(axis=0)  # uniform SPMD
    np.maximum(CH, 1, out=CH)
    CHTOT = int(CH.sum())
    tile_base = np.concatenate([[0], np.cumsum(CH)])[:-1]

    per_core = []
    for c in range(NCORES):
        m = core_of == c
        dsc = ds[m]
        src_r = trow[m]
        dloc = dsc - c * NPC
        tl = dloc // 128
        dl = dloc % 128

        idx = np.zeros((128, CHTOT), dtype=np.int32)  # pad -> row MID (safe)
        dstloc = np.full((128, CHTOT), -1.0, dtype=np.float32)

        # rank of edge within its tile group (stable dst-sorted order)
        korder = np.argsort(tl, kind="stable")
        ks = tl[korder]
        grp_start = np.searchsorted(ks, np.arange(NTILES))
        rank = np.arange(len(ks)) - grp_start[ks]
        chunk = tile_base[ks] + rank // 128
        pos = rank % 128
        idx[pos, chunk] = src_r[korder]
        dstloc[pos, chunk] = dl[korder].astype(np.float32)

        # dma_gather ucode drops TRAILING negative idxs per call; rebased
        # idxs are negative for the lower table half. Ensure the last slot
        # of every call batch is >= 0 by swapping within the batch.
        for t in range(NTILES):
            b0 = tile_base[t]
            done = 0
            while done < CH[t]:
                take = min(CH[t] - done, 8)
                c0 = b0 + done
                clast = c0 + take - 1
                if idx[127, clast] < 0:
                    blk = idx[:, c0 : clast + 1]
                    pp, cc = np.nonzero(blk >= 0)
                    assert len(pp), "all-negative gather call"
                    p2, c2 = pp[0], c0 + cc[0]
                    idx[127, clast], idx[p2, c2] = idx[p2, c2], idx[127, clast]
                    dstloc[127, clast], dstloc[p2, c2] = (
                        dstloc[p2, c2],
                        dstloc[127, clast],
                    )
                done += take

        # idx16: flat slot i = chunk*128 + p -> tile (i%16, i//16)
        flat = idx.T.reshape(-1)
        assert flat.min() >= -32768 and flat.max() < 32768
        idx16 = flat.astype(np.int16).reshape(-1, 16).T.copy()

        per_core.append(
            {
                "idx16": idx16,
                "dstloc": dstloc,
                "dis": dis[c * NPC : (c + 1) * NPC],
                "sprime": sprime[c * NPC : (c + 1) * NPC],
            }
        )
    return per_core, CH, CHTOT, dis


def _build(structure):
    """Build the SPMD Bass program. structure = tuple(CH)."""
    key = structure
    if key in _BUILD_CACHE:
        return _BUILD_CACHE[key]

    import concourse.bass as bass
    import concourse.bacc as bacc
    import concourse.mybir as mybir
    import concourse.tile as tile

    CH = list(structure)
    CHTOT = sum(CH)
    maxch = max(CH)

    nc = bacc.Bacc(
        None, target_bir_lowering=False, debug=False, num_swdge_queues=NQ
    )
    x_in = nc.dram_tensor("x", [NPC, C, L], mybir.dt.bfloat16, kind="ExternalInput")
    dsc_in = nc.dram_tensor("dscale", [128, NTILES], mybir.dt.float32, kind="ExternalInput")
    disv_in = nc.dram_tensor("disv", [128, NTILES], mybir.dt.float32, kind="ExternalInput")
    sp_in = nc.dram_tensor("sp", [1, NPADC], mybir.dt.float32, kind="ExternalInput")
    idx_in = nc.dram_tensor("idx", [16, CHTOT * 8], mybir.dt.int16, kind="ExternalInput")
    dl_in = nc.dram_tensor("dstloc", [128, CHTOT, 1], mybir.dt.bfloat16, kind="ExternalInput")
    iota_in = nc.dram_tensor("iota", [128, 8, 128], mybir.dt.bfloat16, kind="ExternalInput")
    w_in = nc.dram_tensor("W", [C, F], mybir.dt.float32, kind="ExternalInput")
    b_in = nc.dram_tensor("b", [1, F], mybir.dt.float32, kind="ExternalInput")
    out = nc.dram_tensor("out", [NPADC, F], mybir.dt.float32, kind="ExternalOutput")

    with tile.TileContext(nc) as tc:
        with (
            tc.tile_pool(name="sb", bufs=2) as sb,
            tc.tile_pool(name="sbg", bufs=6) as sbg,
            tc.tile_pool(name="sbm", bufs=6) as sbm,
            tc.tile_pool(name="sbx", bufs=6) as sbx,
            tc.tile_pool(name="ps", bufs=4, space="PSUM") as ps,
            tc.tile_pool(name="pso", bufs=4, space="PSUM") as pso,
            tc.tile_pool(name="dram", bufs=1, space="DRAM") as dram,
        ):
            y_loc = dram.tile([NPADC, C], mybir.dt.bfloat16)
            y_full = dram.tile([NROWS, C], mybir.dt.bfloat16)

            # ---- constants ----
            dsc = sb.tile([128, NTILES], mybir.dt.float32, tag="dsc")
            nc.sync.dma_start(dsc[:], dsc_in[:])
            disv = sb.tile([128, NTILES], mybir.dt.float32, tag="disv")
            nc.sync.dma_start(disv[:], disv_in[:])
            spv = sb.tile([1, NPADC], mybir.dt.float32, tag="spv")
            nc.sync.dma_start(spv[:], sp_in[:])
            spb = sb.tile([1, NPADC], mybir.dt.bfloat16, tag="spb")
            nc.vector.tensor_copy(spb[:], spv[:])
            iot8 = sb.tile([128, 8, 128], mybir.dt.bfloat16, tag="iot8")
            nc.sync.dma_start(iot8[:], iota_in[:])
            w32 = sb.tile([128, F], mybir.dt.float32, tag="w32")
            nc.sync.dma_start(w32[:], w_in[:])
            wb = sb.tile([128, F], mybir.dt.bfloat16, tag="wb")
            nc.vector.tensor_copy(wb[:], w32[:])
            b32 = sb.tile([1, F], mybir.dt.float32, tag="b32")
            nc.sync.dma_start(b32[:], b_in[:])
            bb = sb.tile([1, F], mybir.dt.bfloat16, tag="bb")
            nc.vector.tensor_copy(bb[:], b32[:])

            # ---- phase 3 index tables (loaded early, independent of AG) ----
            it = sb.tile([128, CHTOT * 8], mybir.dt.int16, tag="it")
            for k in range(8):
                nc.gpsimd.dma_start(it[16 * k : 16 * (k + 1), :], idx_in[:])
            dltb = sb.tile([128, CHTOT, 1], mybir.dt.bfloat16, tag="dltb")
            nc.gpsimd.dma_start(dltb[:], dl_in[:])

            # ---- phase 1: mean over L, scale, write y_local ----
            for t in range(NTILES):
                n0 = t * 128
                n1 = min(NPC, n0 + 128)
                nn = n1 - n0
                xt = sbx.tile([128, C, L], mybir.dt.bfloat16, tag="xt")
                dma_eng = (nc.sync, nc.gpsimd, nc.scalar, nc.gpsimd)[t % 4]
                dma_eng.dma_start(xt[:nn], x_in[n0:n1])
                xs = sb.tile([128, C], mybir.dt.float32, tag="xs")
                nc.vector.reduce_sum(
                    xs[:nn], xt[:nn], axis=mybir.AxisListType.X,
                )
                yb = sb.tile([128, C], mybir.dt.bfloat16, tag="yb")
                if nn < 128:
                    nc.vector.memset(yb[:], 0.0)
                nc.vector.tensor_scalar_mul(yb[:nn], xs[:nn], dsc[:nn, t : t + 1])
                (nc.sync if t % 2 == 0 else nc.scalar).dma_start(
                    y_loc[n0 : n0 + 128], yb[:]
                )
                if t == 24:
                    nc.gpsimd.collective_compute(
                        "AllGather",
                        mybir.AluOpType.bypass,
                        replica_groups=[list(range(NCORES))],
                        ins=[y_loc[0:SPLIT].opt()],
                        outs=[y_full[0:LO_ROWS, :].opt()],
                    )

            # ---- phase 2: AllGather (second half; first half issued
            # mid-phase-1 above) ----
            nc.gpsimd.collective_compute(
                "AllGather",
                mybir.AluOpType.bypass,
                replica_groups=[list(range(NCORES))],
                ins=[y_loc[SPLIT:NPADC].opt()],
                outs=[y_full[LO_ROWS:, :].opt()],
            )

            # ---- phase 3: gather + one-hot scatter matmuls ----
            qrr = 0
            chbase = 0
            for t in range(NTILES):
                ctot = CH[t]
                pt = ps.tile([128, 128], mybir.dt.float32)
                done = 0
                while done < ctot:
                    take = min(ctot - done, 8)
                    c0 = chbase + done
                    gb = sbg.tile([128, 8, C], mybir.dt.bfloat16, tag="g")
                    nc.gpsimd.dma_gather(
                        gb[:, :take, :],
                        y_full[MID:, :],
                        it[:, c0 * 8 : (c0 + take) * 8],
                        take * 128,
                        take * 128,
                        C,
                        queue_num=qrr % NQ,
                    )
                    qrr += 1
                    mb = sbm.tile([128, 8, 128], mybir.dt.bfloat16, tag="m")
                    nc.vector.tensor_tensor(
                        out=mb[:, :take, :],
                        in0=iot8[:, :take, :],
                        in1=dltb[:, c0 : c0 + take, :].to_broadcast([128, take, 128]),
                        op=mybir.AluOpType.is_equal,
                    )
                    for ch in range(take):
                        nc.tensor.matmul(
                            pt[:],
                            gb[:, ch, :],
                            mb[:, ch, :],
                            start=(done + ch == 0),
                            stop=(done + ch == ctot - 1),
                        )
                    done += take
                aggb = sb.tile([128, 128], mybir.dt.bfloat16, tag="aggb")
                nc.scalar.copy(aggb[:], pt[:])
                po = pso.tile([128, F], mybir.dt.float32)
                nc.tensor.matmul(po[:], aggb[:], wb[:], start=True, stop=False)
                nc.tensor.matmul(
                    po[:],
                    spb[:, t * 128 : (t + 1) * 128],
                    bb[:],
                    start=False,
                    stop=True,
                )
                ot = sb.tile([128, F], mybir.dt.float32, tag="ot")
                nc.scalar.mul(ot[:], po[:], disv[:, t : t + 1])
                nc.sync.dma_start(out[t * 128 : (t + 1) * 128], ot[:])
                chbase += ctot

    nc.finalize()
    _BUILD_CACHE[key] = nc
    return nc


def kernel(x, edge_index, W, b):
    x = np.ascontiguousarray(np.asarray(x, dtype=np.float32))
    edge_index = np.asarray(edge_index)
    W = np.ascontiguousarray(np.asarray(W, dtype=np.float32))
    b = np.ascontiguousarray(np.asarray(b, dtype=np.float32))

    per_core, CH, CHTOT, dis = _preprocess(edge_index)
    nc = _build(tuple(int(v) for v in CH))

    import ml_dtypes

    iota = np.broadcast_to(
        np.arange(128, dtype=np.float32), (128, 8, 128)
    ).astype(ml_dtypes.bfloat16)
    in_maps = []
    for c in range(NCORES):
        pc = per_core[c]
        dpad = np.zeros(NPADC, dtype=np.float32)
        dpad[:NPC] = pc["dis"]
        spad = np.zeros(NPADC, dtype=np.float32)
        spad[:NPC] = pc["sprime"]
        dsc = np.ascontiguousarray((dpad / L).reshape(NTILES, 128).T)
        dsv = np.ascontiguousarray(dpad.reshape(NTILES, 128).T)
        in_maps.append(
            {
                "x": np.ascontiguousarray(
                    x[c * NPC : (c + 1) * NPC].transpose(0, 2, 1)
                ).astype(ml_dtypes.bfloat16),
                "dscale": dsc,
                "disv": dsv,
                "sp": spad.reshape(1, NPADC),
                "idx": pc["idx16"],
                "dstloc": pc["dstloc"].astype(ml_dtypes.bfloat16)[:, :, None],
                "iota": iota,
                "W": W,
                "b": b.reshape(1, F),
            }
        )

    from concourse.bass_utils import run_bass_kernel_spmd

    res = run_bass_kernel_spmd(nc, in_maps, core_ids=list(range(NCORES)))
    out = np.empty((N, F), dtype=np.float32)
    for c in range(NCORES):
        out[c * NPC : (c + 1) * NPC] = res.results[c]["out"][:NPC]
    return out


# revision 12
# speedup vs baseline: 1.3150x; 1.1479x over previous
"""GCN layer kernel for nn_GcnNet_17695265259748 — full on-device Bass SPMD.

Pipeline per NeuronCore (8 cores, nodes sharded 6250/core):
  1. stream x shard [6250,20,128] f32, reduce over L on DVE, scale by
     dis/L -> y_local bf16 [6272,128] (22 zero pad rows) in DRAM
  2. AllGather -> y_full [50176,128] bf16 (all cores' scaled features)
  3. per dst tile (49 x 128 dsts): dma_gather edge source rows across the
     4 SWDGE queues (signed idx16 rebased at mid-table so one table covers
     all 50176 rows), one-hot M built on DVE in batches (broadcast
     tensor_tensor, bf16), PSUM aggT += G.T @ M per chunk
  4. proj: psum_out = aggT.T @ W + outer(s', b);  out = dis * psum_out
Host does edge preprocessing (sort by dst, chunking, index tables).
"""

import sys
import numpy as np

for p in ("/opt/trn_rl_repo",):
    if p not in sys.path:
        sys.path.insert(0, p)

N, L, C, F = 50000, 20, 128, 300
NCORES = 8
NPC = N // NCORES            # 6250 nodes per core
NTILES = (NPC + 127) // 128  # 49 dst tiles (last partial: 106)
NPADC = NTILES * 128         # 6272 rows per core chunk in the table
NROWS = NCORES * NPADC       # 50176 table rows
MID = NROWS // 2             # 25088: signed-idx16 rebase point
MAXCALL = 1024               # dma_gather ring limit (rows per call)
NQ = 4                       # SWDGE queues (gpsimd cpu pairs)

_BUILD_CACHE = {}


SPLIT = 3200            # phase-1 row boundary for the two AllGather halves
LO_ROWS = NCORES * SPLIT  # 25600


def _table_row(src):
    """Row in y_full given the two-block AllGather layout:
    lo block = concat over cores of y_loc[0:SPLIT], hi block = rest."""
    core = src // NPC
    r = src % NPC
    return np.where(
        r < SPLIT,
        core * SPLIT + r,
        LO_ROWS + core * (NPADC - SPLIT) + (r - SPLIT),
    )


def _preprocess(edge_index):
    """Per-core idx/dstloc tables + uniform per-tile chunk counts."""
    row = np.ascontiguousarray(edge_index[0]).astype(np.int64)
    col = np.ascontiguousarray(edge_index[1]).astype(np.int64)
    keep = row != col
    loops = np.arange(N, dtype=np.int64)
    srcs = np.concatenate([row[keep], loops])
    dsts = np.concatenate([col[keep], loops])

    deg = np.bincount(row[keep], minlength=N).astype(np.float64) + 1.0
    dis = (deg ** -0.5).astype(np.float32)
    sprime = np.bincount(dsts, weights=dis[srcs].astype(np.float64), minlength=N)
    sprime = sprime.astype(np.float32)

    order = np.argsort(dsts, kind="stable")
    ds = dsts[order]
    sr = srcs[order]
    trow = _table_row(sr) - MID  # rebased signed index, in [-25088, 25087]

    core_of = ds // NPC
    tloc = (ds - core_of * NPC) // 128
    gt = core_of * NTILES + tloc
    cnt = np.bincount(gt, minlength=NCORES * NTILES)
    CH = ((cnt.reshape(NCORES, NTILES) + 127) // 128).max(axis=0)  # uniform SPMD
    np.maximum(CH, 1, out=CH)
    CHTOT = int(CH.sum())
    tile_base = np.concatenate([[0], np.cumsum(CH)])[:-1]

    per_core = []
    for c in range(NCORES):
        m = core_of == c
        dsc = ds[m]
        src_r = trow[m]
        dloc = dsc - c * NPC
        tl = dloc // 128
        dl = dloc % 128

        idx = np.zeros((128, CHTOT), dtype=np.int32)  # pad -> row MID (safe)
        dstloc = np.full((128, CHTOT), -1.0, dtype=np.float32)

        # rank of edge within its tile group (stable dst-sorted order)
        korder = np.argsort(tl, kind="stable")
        ks = tl[korder]
        grp_start = np.searchsorted(ks, np.arange(NTILES))
        rank = np.arange(len(ks)) - grp_start[ks]
        chunk = tile_base[ks] + rank // 128
        pos = rank % 128
        idx[pos, chunk] = src_r[korder]
        dstloc[pos, chunk] = dl[korder].astype(np.float32)

        # dma_gather ucode drops TRAILING negative idxs per call; rebased
        # idxs are negative for the lower table half. Ensure the last slot
        # of every call batch is >= 0 by swapping within the batch.
        for t in range(NTILES):
            b0 = tile_base[t]
            done = 0
            while done < CH[t]:
                take = min(CH[t] - done, 8)
                c0 = b0 + done
                clast = c0 + take - 1
                if idx[127, clast] < 0:
                    blk = idx[:, c0 : clast + 1]
                    pp, cc = np.nonzero(blk >= 0)
                    assert len(pp), "all-negative gather call"
                    p2, c2 = pp[0], c0 + cc[0]
                    idx[127, clast], idx[p2, c2] = idx[p2, c2], idx[127, clast]
                    dstloc[127, clast], dstloc[p2, c2] = (
                        dstloc[p2, c2],
                        dstloc[127, clast],
                    )
                done += take

        # idx16: flat slot i = chunk*128 + p -> tile (i%16, i//16)
        flat = idx.T.reshape(-1)
        assert flat.min() >= -32768 and flat.max() < 32768
        idx16 = flat.astype(np.int16).reshape(-1, 16).T.copy()

        per_core.append(
            {
                "idx16": idx16,
                "dstloc": dstloc,
                "dis": dis[c * NPC : (c + 1) * NPC],
                "sprime": sprime[c * NPC : (c + 1) * NPC],
            }
        )
    return per_core, CH, CHTOT, dis


def _build(structure):
    """Build the SPMD Bass program. structure = tuple(CH)."""
    key = structure
    if key in _BUILD_CACHE:
        return _BUILD_CACHE[key]

    import concourse.bass as bass
    import concourse.bacc as bacc
    import concourse.mybir as mybir
    import concourse.tile as tile

    CH = list(structure)
    CHTOT = sum(CH)
    maxch = max(CH)

    nc = bacc.Bacc(
        None, target_bir_lowering=False, debug=False, num_swdge_queues=NQ
    )
    x_in = nc.dram_tensor("x", [NPC, C, L], mybir.dt.bfloat16, kind="ExternalInput")
    dsc_in = nc.dram_tensor("dscale", [128, NTILES], mybir.dt.float32, kind="ExternalInput")
    disv_in = nc.dram_tensor("disv", [128, NTILES], mybir.dt.float32, kind="ExternalInput")
    sp_in = nc.dram_tensor("sp", [1, NPADC], mybir.dt.float32, kind="ExternalInput")
    idx_in = nc.dram_tensor("idx", [16, CHTOT * 8], mybir.dt.int16, kind="ExternalInput")
    dl_in = nc.dram_tensor("dstloc", [128, CHTOT, 1], mybir.dt.bfloat16, kind="ExternalInput")
    iota_in = nc.dram_tensor("iota", [128, 8, 128], mybir.dt.bfloat16, kind="ExternalInput")
    w_in = nc.dram_tensor("W", [C, F], mybir.dt.float32, kind="ExternalInput")
    b_in = nc.dram_tensor("b", [1, F], mybir.dt.float32, kind="ExternalInput")
    out = nc.dram_tensor("out", [NPADC, F], mybir.dt.float32, kind="ExternalOutput")

    with tile.TileContext(nc) as tc:
        with (
            tc.tile_pool(name="sb", bufs=2) as sb,
            tc.tile_pool(name="cst", bufs=1) as cst,
            tc.tile_pool(name="sbg", bufs=10) as sbg,
            tc.tile_pool(name="sbm", bufs=10) as sbm,
            tc.tile_pool(name="sbx", bufs=9) as sbx,
            tc.tile_pool(name="ps", bufs=4, space="PSUM") as ps,
            tc.tile_pool(name="pso", bufs=4, space="PSUM") as pso,
            tc.tile_pool(name="dram", bufs=1, space="DRAM") as dram,
        ):
            y_loc = dram.tile([NPADC, C], mybir.dt.bfloat16)
            y_full = dram.tile([NROWS, C], mybir.dt.bfloat16)

            # ---- constants ----
            dsc = cst.tile([128, NTILES], mybir.dt.float32, tag="dsc")
            nc.sync.dma_start(dsc[:], dsc_in[:])
            disv = cst.tile([128, NTILES], mybir.dt.float32, tag="disv")
            nc.sync.dma_start(disv[:], disv_in[:])
            spv = cst.tile([1, NPADC], mybir.dt.float32, tag="spv")
            nc.sync.dma_start(spv[:], sp_in[:])
            spb = cst.tile([1, NPADC], mybir.dt.bfloat16, tag="spb")
            nc.vector.tensor_copy(spb[:], spv[:])
            iot8 = cst.tile([128, 8, 128], mybir.dt.bfloat16, tag="iot8")
            nc.sync.dma_start(iot8[:], iota_in[:])
            w32 = cst.tile([128, F], mybir.dt.float32, tag="w32")
            nc.sync.dma_start(w32[:], w_in[:])
            wb = cst.tile([128, F], mybir.dt.bfloat16, tag="wb")
            nc.vector.tensor_copy(wb[:], w32[:])
            b32 = cst.tile([1, F], mybir.dt.float32, tag="b32")
            nc.sync.dma_start(b32[:], b_in[:])
            bb = cst.tile([1, F], mybir.dt.bfloat16, tag="bb")
            nc.vector.tensor_copy(bb[:], b32[:])

            # ---- phase 3 index tables (loaded early, independent of AG) ----
            it = cst.tile([128, CHTOT * 8], mybir.dt.int16, tag="it")
            for k in range(8):
                nc.gpsimd.dma_start(it[16 * k : 16 * (k + 1), :], idx_in[:])
            dltb = cst.tile([128, CHTOT, 1], mybir.dt.bfloat16, tag="dltb")
            nc.gpsimd.dma_start(dltb[:], dl_in[:])

            # ---- phase 1: mean over L, scale, write y_local ----
            for t in range(NTILES):
                n0 = t * 128
                n1 = min(NPC, n0 + 128)
                nn = n1 - n0
                xt = sbx.tile([128, C, L], mybir.dt.bfloat16, tag="xt")
                dma_eng = (nc.sync, nc.gpsimd, nc.scalar)[t % 3]
                dma_eng.dma_start(xt[:nn], x_in[n0:n1])
                xs = sb.tile([128, C], mybir.dt.float32, tag="xs")
                nc.vector.reduce_sum(
                    xs[:nn], xt[:nn], axis=mybir.AxisListType.X,
                )
                yb = sb.tile([128, C], mybir.dt.bfloat16, tag="yb")
                if nn < 128:
                    nc.vector.memset(yb[:], 0.0)
                nc.vector.tensor_scalar_mul(yb[:nn], xs[:nn], dsc[:nn, t : t + 1])
                (nc.sync if t % 2 == 0 else nc.scalar).dma_start(
                    y_loc[n0 : n0 + 128], yb[:]
                )
                if t == 24:
                    nc.gpsimd.collective_compute(
                        "AllGather",
                        mybir.AluOpType.bypass,
                        replica_groups=[list(range(NCORES))],
                        ins=[y_loc[0:SPLIT].opt()],
                        outs=[y_full[0:LO_ROWS, :].opt()],
                    )

            # ---- phase 2: AllGather (second half; first half issued
            # mid-phase-1 above) ----
            nc.gpsimd.collective_compute(
                "AllGather",
                mybir.AluOpType.bypass,
                replica_groups=[list(range(NCORES))],
                ins=[y_loc[SPLIT:NPADC].opt()],
                outs=[y_full[LO_ROWS:, :].opt()],
            )

            # ---- phase 3: gather + one-hot scatter matmuls ----
            qrr = 0
            chbase = 0
            for t in range(NTILES):
                ctot = CH[t]
                pt = ps.tile([128, 128], mybir.dt.float32)
                done = 0
                while done < ctot:
                    take = min(ctot - done, 8)
                    c0 = chbase + done
                    gb = sbg.tile([128, 8, C], mybir.dt.bfloat16, tag="g")
                    nc.gpsimd.dma_gather(
                        gb[:, :take, :],
                        y_full[MID:, :],
                        it[:, c0 * 8 : (c0 + take) * 8],
                        take * 128,
                        take * 128,
                        C,
                        queue_num=qrr % NQ,
                    )
                    qrr += 1
                    mb = sbm.tile([128, 8, 128], mybir.dt.bfloat16, tag="m")
                    nc.vector.tensor_tensor(
                        out=mb[:, :take, :],
                        in0=iot8[:, :take, :],
                        in1=dltb[:, c0 : c0 + take, :].to_broadcast([128, take, 128]),
                        op=mybir.AluOpType.is_equal,
                    )
                    for ch in range(take):
                        nc.tensor.matmul(
                            pt[:],
                            gb[:, ch, :],
                            mb[:, ch, :],
                            start=(done + ch == 0),
                            stop=(done + ch == ctot - 1),
                        )
                    done += take
                aggb = sb.tile([128, 128], mybir.dt.bfloat16, tag="aggb")
                nc.scalar.copy(aggb[:], pt[:])
                po = pso.tile([128, F], mybir.dt.float32)
                nc.tensor.matmul(po[:], aggb[:], wb[:], start=True, stop=False)
                nc.tensor.matmul(
                    po[:],
                    spb[:, t * 128 : (t + 1) * 128],
                    bb[:],
                    start=False,
                    stop=True,
                )
                ot = sb.tile([128, F], mybir.dt.float32, tag="ot")
                nc.scalar.mul(ot[:], po[:], disv[:, t : t + 1])
                nc.sync.dma_start(out[t * 128 : (t + 1) * 128], ot[:])
                chbase += ctot

    nc.finalize()
    _BUILD_CACHE[key] = nc
    return nc


def kernel(x, edge_index, W, b):
    x = np.ascontiguousarray(np.asarray(x, dtype=np.float32))
    edge_index = np.asarray(edge_index)
    W = np.ascontiguousarray(np.asarray(W, dtype=np.float32))
    b = np.ascontiguousarray(np.asarray(b, dtype=np.float32))

    per_core, CH, CHTOT, dis = _preprocess(edge_index)
    nc = _build(tuple(int(v) for v in CH))

    import ml_dtypes

    iota = np.broadcast_to(
        np.arange(128, dtype=np.float32), (128, 8, 128)
    ).astype(ml_dtypes.bfloat16)
    in_maps = []
    for c in range(NCORES):
        pc = per_core[c]
        dpad = np.zeros(NPADC, dtype=np.float32)
        dpad[:NPC] = pc["dis"]
        spad = np.zeros(NPADC, dtype=np.float32)
        spad[:NPC] = pc["sprime"]
        dsc = np.ascontiguousarray((dpad / L).reshape(NTILES, 128).T)
        dsv = np.ascontiguousarray(dpad.reshape(NTILES, 128).T)
        in_maps.append(
            {
                "x": np.ascontiguousarray(
                    x[c * NPC : (c + 1) * NPC].transpose(0, 2, 1)
                ).astype(ml_dtypes.bfloat16),
                "dscale": dsc,
                "disv": dsv,
                "sp": spad.reshape(1, NPADC),
                "idx": pc["idx16"],
                "dstloc": pc["dstloc"].astype(ml_dtypes.bfloat16)[:, :, None],
                "iota": iota,
                "W": W,
                "b": b.reshape(1, F),
            }
        )

    from concourse.bass_utils import run_bass_kernel_spmd

    res = run_bass_kernel_spmd(nc, in_maps, core_ids=list(range(NCORES)))
    out = np.empty((N, F), dtype=np.float32)
    for c in range(NCORES):
        out[c * NPC : (c + 1) * NPC] = res.results[c]["out"][:NPC]
    return out
